# revision 18
# baseline (speedup 1.0000x reference)
"""Trainium2 Bass kernel for nn_CnUpdateLayer (LDPC check-node update).

Math: out[b,i] = prod_{j: mask[i,j]!=0} x[b,j], or 0 if mask row i is empty.
Since mask is exactly {0,1} and x ~ randn (no exact zeros), the masked product
is computed in log-domain via matmul:

    L[b,i]  = sum_j ln|x[b,j]| * mask[i,j]       (magnitude, log domain)
    C[b,i]  = sum_j [x[b,j]<0] * mask[i,j]       (negative count)
    deg[i]  = sum_j mask[i,j]                    (row degree)
    out     = exp(.5*Lhi)*exp(.5*Llo) * (min(deg,1) - 2*(C&1))

ln(x^2) is split hi/lo into two bf16 halves so the matmul runs at bf16 rate
while keeping ~fp32 accuracy.  The stationary operand is
[ln_hi | ln_lo | signbits | ones*32] = 128 columns: the 32 identical "ones"
columns replicate the row degree into PSUM partitions 96:128, so the epilogue
needs no K=1 broadcast matmul.

Schedule (v3):
  - x (and the pre-extracted sign bits) ride the scalar HWDGE ring; the full
    mask streams on the sync HWDGE ring in 4 groups whose arrival order
    matches matmul chunk order.  GpSimd is not used (its ALU is ~30x slower
    than DVE and its SBUF traffic interferes with DVE).
  - The PE is kept busy with dummy matmuls from preamble-end until the real
    accumulation starts, so the real matmuls run at the 2.4GHz hot p-state.
  - ln-prep: 4 pipelined blocks; DVE does squares + hi-casts + lo, ACT only
    the 4 Ln's.
  - Epilogue: one ACT copy moves all 128 PSUM rows to SBUF (PSUM readers
    serialize per bank, so one reader beats four); parity ops run at
    partition base 96 so every instruction's SBUF inputs share a base.
"""

import sys

if "/opt/trn_rl_repo" not in sys.path:
    sys.path.insert(0, "/opt/trn_rl_repo")

import numpy as np

B = 32          # batch codewords
IN_F = 2048     # input edges
OUT_F = 2048    # output edges
NCORES = 8
SHARD = OUT_F // NCORES     # 256 output edges per core
KC = IN_F // 128            # 16 contraction chunks of 128
PB = 4                      # prep block size (chunks)
WHI, WLO, WSGN, WONE = 0, B, 2 * B, 3 * B       # 0, 32, 64, 96
WTOT = 4 * B                                    # 128

NDUM_BIG = 12           # 512-row dummy matmuls (PE p-state warmers)
NDUM_SM = 6             # 64-row dummy matmuls (fine tail trim)

_PROG = None


def _build_program():
    import concourse.tile as tile
    from concourse import bacc, mybir
    from concourse.alu_op_type import AluOpType

    F32 = mybir.dt.float32
    F16 = mybir.dt.float16
    I32 = mybir.dt.int32
    BF16 = mybir.dt.bfloat16
    AF = mybir.ActivationFunctionType

    nc = bacc.Bacc("TRN2", target_bir_lowering=False)
    xt = nc.dram_tensor("xt", [128, KC * B], F16, kind="ExternalInput")
    st = nc.dram_tensor("st", [128, KC * B], BF16, kind="ExternalInput")
    mt = nc.dram_tensor("mt", [128, KC * SHARD], BF16, kind="ExternalInput")
    out = nc.dram_tensor("out", [B, SHARD], F32, kind="ExternalOutput")

    with tile.TileContext(nc) as tc:
        with (
            tc.tile_pool(name="pool", bufs=1) as pool,
            tc.tile_pool(name="psum", bufs=1, space="PSUM") as psum_pool,
        ):
            # ---- input DMAs.  x + sign bits on the scalar HWDGE ring (ahead
            # of the Ln table load, which overlaps their transfer); the mask
            # alone on the sync ring in 4 groups of 4 chunks so arrival order
            # matches matmul consumption order.
            # dummy Ln first on the scalar queue: its 1.28us ACT_TABLE_LOAD
            # runs while x streams on the sync ring, and naturally delays the
            # scalar-ring mask issues so x gets the full DMA bandwidth first.
            dmy = pool.tile([1, 1], F32)
            nc.vector.memset(dmy, 1.0)
            dln = pool.tile([1, 1], F32)
            nc.scalar.activation(out=dln, in_=dmy, func=AF.Ln)

            x_sb = pool.tile([128, B, KC], F16)
            nc.sync.dma_start(
                out=x_sb, in_=xt.ap().rearrange("p (b c) -> p b c", c=KC))
            w_sb = pool.tile([128, WTOT, KC], BF16)
            nc.sync.dma_start(
                out=w_sb[:, WSGN:WSGN + B, :],
                in_=st.ap().rearrange("p (b c) -> p b c", c=KC))
            m_sb = pool.tile([128, KC, SHARD], BF16)
            mt_v = mt.ap().rearrange("p (c n) -> p c n", n=SHARD)
            nc.gpsimd.dma_start(out=m_sb[:, 0:4, :], in_=mt_v[:, 0:4, :])
            nc.gpsimd.dma_start(out=m_sb[:, 4:8, :], in_=mt_v[:, 4:8, :])
            nc.scalar.dma_start(out=m_sb[:, 8:12, :], in_=mt_v[:, 8:12, :])
            nc.sync.dma_start(out=m_sb[:, 12:16, :], in_=mt_v[:, 12:16, :])

            # the replicated-ones block of W: DVE memset, no deps,
            # scheduled right after the preamble.
            nc.vector.memset(w_sb[:, WONE:WONE + B, :], 1.0)

            # short K=1 dummy-matmul bridge: keeps the PE awake (out of the
            # cold p-state) until the first mask group + W block land, with
            # negligible SBUF traffic.  (The hot 2.4GHz state is unreachable
            # in this kernel's ~3us windows - HW DVFS needs sustained real
            # utilization - so the bridge only needs to cover, not warm.)
            wdum = pool.tile([1, 1], BF16)
            nc.vector.memset(wdum, 0.0)
            mdum = pool.tile([1, 512], BF16)
            nc.vector.memset(mdum, 0.0)
            ps_dum = psum_pool.tile([1, 512], F32)
            for _ in range(7):
                nc.tensor.matmul(ps_dum, lhsT=wdum, rhs=mdum, start=True, stop=True)
            for _ in range(4):
                nc.tensor.matmul(ps_dum[:, 0:64], lhsT=wdum, rhs=mdum[:, 0:64],
                                 start=True, stop=True)

            # ---- stationary operand W = [hi | lo | sgn | ones], bf16, in 4
            # pipelined blocks of 4 chunks.  ln|x| = ln(x^2) (x^2 on DVE
            # avoids the Abs table); the 0.5 is folded into the Exp scale.
            # ACT does only the Ln's; DVE everything else.
            sq_sb = pool.tile([128, B, KC], F32)
            ln_sb = pool.tile([128, B, KC], F32)
            for h in range(0, KC, PB):
                sl = slice(h, h + PB)
                nc.vector.tensor_tensor(
                    out=sq_sb[:, :, sl], in0=x_sb[:, :, sl], in1=x_sb[:, :, sl],
                    op=AluOpType.mult)
                nc.scalar.activation(out=ln_sb[:, :, sl], in_=sq_sb[:, :, sl], func=AF.Ln)
                nc.vector.tensor_scalar(
                    out=w_sb[:, WHI:WHI + B, sl], in0=ln_sb[:, :, sl],
                    scalar1=0.0, scalar2=None, op0=AluOpType.add)
                nc.vector.tensor_tensor(
                    out=w_sb[:, WLO:WLO + B, sl], in0=ln_sb[:, :, sl],
                    in1=w_sb[:, WHI:WHI + B, sl], op=AluOpType.subtract)

            # dummy Exp AFTER the Ln phase (input reads ln_sb to pin the
            # ordering): its table load overlaps the matmuls instead of
            # stalling the real Exps.
            dex = pool.tile([1, 1], F32)
            nc.scalar.activation(out=dex, in_=ln_sb[0:1, 0:1, KC - 1], func=AF.Exp)

            # ---- main accumulation: ps[0:128] += W_c^T @ M_c over 16 chunks.
            # Rows 0:32 = Lhi, 32:64 = Llo, 64:96 = C, 96:128 = deg (x32).
            ps = psum_pool.tile([128, SHARD], F32)
            for c in range(KC):
                nc.tensor.matmul(
                    ps, lhsT=w_sb[:, :, c], rhs=m_sb[:, c, :],
                    start=(c == 0), stop=(c == KC - 1))

            # ---- epilogue.  One ACT copy moves the whole PSUM bank to SBUF
            # (PSUM readers serialize per bank - one reader beats four), then:
            #   magh/magl = exp(.5 * csb[0:32 / 32:64])          (ACT)
            #   ci2  = 2*C as int32            (csb[64:96] -> base 96)
            #   odd2 = ci2 & 2                 (base 96)
            #   zv   = min(deg,1) - odd2       (csb[96:128] & odd2, base 96 ->
            #                                   base 0; only INPUT bases must
            #                                   match for 2-tensor DVE ops)
            #   out  = magh*magl*zv
            # deg==0 implies C==0, so empty rows get exactly 0.
            csb = pool.tile([128, SHARD], F32)
            nc.scalar.copy(out=csb, in_=ps)
            ci2 = pool.tile([128, SHARD], I32)
            nc.vector.tensor_scalar(
                out=ci2[WONE:WONE + B, :], in0=csb[WSGN:WSGN + B, :],
                scalar1=2.0, scalar2=None, op0=AluOpType.mult)
            odd2 = pool.tile([128, SHARD], I32)
            nc.vector.tensor_scalar(
                out=odd2[WONE:WONE + B, :], in0=ci2[WONE:WONE + B, :],
                scalar1=2, scalar2=None, op0=AluOpType.bitwise_and)
            zv = pool.tile([B, SHARD], F32)
            nc.vector.scalar_tensor_tensor(
                out=zv, in0=csb[WONE:WONE + B, :], scalar=1.0,
                in1=odd2[WONE:WONE + B, :],
                op0=AluOpType.min, op1=AluOpType.subtract)
            magh = pool.tile([B, SHARD], F32)
            nc.scalar.activation(out=magh, in_=csb[WHI:WHI + B, :], func=AF.Exp, scale=0.5)
            magl = pool.tile([B, SHARD], F32)
            nc.scalar.activation(out=magl, in_=csb[WLO:WLO + B, :], func=AF.Exp, scale=0.5)
            a = pool.tile([B, SHARD], F32)
            nc.vector.tensor_tensor(out=a, in0=magh, in1=magl, op=AluOpType.mult)
            o_sb = pool.tile([B, SHARD], F32)
            nc.vector.tensor_tensor(out=o_sb, in0=a, in1=zv, op=AluOpType.mult)
            # output in two halves on the two HWDGE rings
            H = SHARD // 2
            nc.sync.dma_start(out=out.ap()[:, 0:H], in_=o_sb[:, 0:H])
            nc.scalar.dma_start(out=out.ap()[:, H:SHARD], in_=o_sb[:, H:SHARD])

    nc.compile()
    return nc


def _get_program():
    global _PROG
    if _PROG is None:
        _PROG = _build_program()
    return _PROG


def _prep_inputs(x, mask):
    import ml_dtypes

    x = np.ascontiguousarray(x, dtype=np.float32)
    mask = np.ascontiguousarray(mask, dtype=np.float32)
    # xt[p, b*KC + c] = x[b, c*128 + p]
    xtf = np.ascontiguousarray(
        x.T.reshape(KC, 128, B).transpose(1, 2, 0).reshape(128, B * KC))
    st = (xtf < 0).astype(ml_dtypes.bfloat16)
    xt = xtf.astype(np.float16)
    mask_bf = mask.astype(ml_dtypes.bfloat16)
    in_maps = []
    for k in range(NCORES):
        shard = mask_bf[k * SHARD:(k + 1) * SHARD, :]      # [256, 2048]
        # mt[p, c*SHARD + n] = mask[k*SHARD + n, c*128 + p]
        mt = np.ascontiguousarray(
            shard.T.reshape(KC, 128, SHARD).transpose(1, 0, 2).reshape(128, KC * SHARD))
        in_maps.append({"xt": xt, "st": st, "mt": mt})
    return in_maps


def run(x, mask, trace=False):
    """Run on 8 NeuronCores; returns (output, BassKernelResults)."""
    from concourse.bass_utils import run_bass_kernel_spmd

    nc = _get_program()
    in_maps = _prep_inputs(x, mask)
    res = run_bass_kernel_spmd(nc, in_maps, core_ids=list(range(NCORES)), trace=trace)
    out = np.concatenate([r["out"] for r in res.results], axis=1)
    return np.ascontiguousarray(out, dtype=np.float32), res


def kernel(x, mask):
    out, _ = run(x, mask, trace=False)
    return out


# revision 20
# speedup vs baseline: 1.0262x; 1.0262x over previous
"""Trainium2 Bass kernel for nn_CnUpdateLayer (LDPC check-node update).

Math: out[b,i] = prod_{j: mask[i,j]!=0} x[b,j], or 0 if mask row i is empty.
Since mask is exactly {0,1} and x ~ randn (no exact zeros), the masked product
is computed in log-domain via matmul:

    L[b,i]  = sum_j ln|x[b,j]| * mask[i,j]       (magnitude, log domain)
    C[b,i]  = sum_j [x[b,j]<0] * mask[i,j]       (negative count)
    deg[i]  = sum_j mask[i,j]                    (row degree)
    out     = exp(.5*Lhi)*exp(.5*Llo) * (min(deg,1) - 2*(C&1))

ln(x^2) is split hi/lo into two bf16 halves so the matmul runs at bf16 rate
while keeping ~fp32 accuracy.  The stationary operand is
[ln_hi | ln_lo | signbits | ones*32] = 128 columns: the 32 identical "ones"
columns replicate the row degree into PSUM partitions 96:128, so the epilogue
needs no K=1 broadcast matmul.

Schedule (final):
  - Inputs ride 3 DMA rings: x (fp16) + sign bits + last mask group on the
    sync HWDGE ring, first two mask groups on the SWDGE (gpsimd) ring (it
    sustains 150-230 GB/s vs ~40-80 for each HWDGE ring when all are live),
    third group on the scalar HWDGE ring.  Mask group arrival order matches
    matmul chunk order.  W is stored column-major [128, 128cols, KC] so the
    sign-bit DMA and ones memset write contiguous bytes per partition (a
    strided 64B-element dest shatters the DMA into tiny packets and chokes
    every ring) while each chunk's lhsT stays a single-stride AP.
  - A short K=1 dummy-matmul bridge keeps the PE out of its cold p-state
    until the first mask group lands (the hot 2.4GHz state is unreachable:
    HW DVFS needs sustained real utilization, so matmuls run at 1.2GHz).
  - ln-prep: 4 pipelined blocks; DVE does squares + hi-casts + lo, ACT only
    the 4 Ln's.  GpSimd ALU is never used (~30x slower than DVE and its
    SBUF traffic interferes with DVE).
  - Epilogue: one ACT copy moves all 128 PSUM rows to SBUF (PSUM readers
    serialize per bank, so one reader beats four); int parity (x2 -> int32,
    &2) runs on DVE at partition base 96 so every 2-tensor instruction's
    SBUF inputs share a base (a hardware requirement); output leaves in two
    halves on the two HWDGE rings.
"""

import sys

if "/opt/trn_rl_repo" not in sys.path:
    sys.path.insert(0, "/opt/trn_rl_repo")

import numpy as np

B = 32          # batch codewords
IN_F = 2048     # input edges
OUT_F = 2048    # output edges
NCORES = 8
SHARD = OUT_F // NCORES     # 256 output edges per core
KC = IN_F // 128            # 16 contraction chunks of 128
PB = 4                      # prep block size (chunks)
WHI, WLO, WSGN, WONE = 0, B, 2 * B, 3 * B       # 0, 32, 64, 96
WTOT = 4 * B                                    # 128

NDUM_BIG = 12           # 512-row dummy matmuls (PE p-state warmers)
NDUM_SM = 6             # 64-row dummy matmuls (fine tail trim)

_PROG = None


def _build_program():
    import concourse.tile as tile
    from concourse import bacc, mybir
    from concourse.alu_op_type import AluOpType

    F32 = mybir.dt.float32
    F16 = mybir.dt.float16
    I32 = mybir.dt.int32
    BF16 = mybir.dt.bfloat16
    AF = mybir.ActivationFunctionType

    nc = bacc.Bacc("TRN2", target_bir_lowering=False)
    xt = nc.dram_tensor("xt", [128, KC * B], F16, kind="ExternalInput")
    I8 = mybir.dt.int8
    st = nc.dram_tensor("st", [128, KC * B], I8, kind="ExternalInput")
    mt = nc.dram_tensor("mt", [128, KC * SHARD], BF16, kind="ExternalInput")
    out = nc.dram_tensor("out", [B, SHARD], F32, kind="ExternalOutput")

    with tile.TileContext(nc) as tc:
        with (
            tc.tile_pool(name="pool", bufs=1) as pool,
            tc.tile_pool(name="psum", bufs=1, space="PSUM") as psum_pool,
        ):
            # ---- input DMAs.  x + sign bits on the scalar HWDGE ring (ahead
            # of the Ln table load, which overlaps their transfer); the mask
            # alone on the sync ring in 4 groups of 4 chunks so arrival order
            # matches matmul consumption order.
            # dummy Ln first on the scalar queue: its 1.28us ACT_TABLE_LOAD
            # runs while x streams on the sync ring, and naturally delays the
            # scalar-ring mask issues so x gets the full DMA bandwidth first.
            dmy = pool.tile([1, 1], F32)
            nc.vector.memset(dmy, 1.0)
            dln = pool.tile([1, 1], F32)
            nc.scalar.activation(out=dln, in_=dmy, func=AF.Ln)

            x_sb = pool.tile([128, B, KC], F16)
            nc.sync.dma_start(
                out=x_sb, in_=xt.ap().rearrange("p (b c) -> p b c", c=KC))
            w_sb = pool.tile([128, WTOT, KC], BF16)
            nc.gpsimd.dma_start(
                out=w_sb[:, WSGN:WSGN + B, :],
                in_=st.ap().rearrange("p (b c) -> p b c", c=KC))
            m_sb = pool.tile([128, KC, SHARD], BF16)
            mt_v = mt.ap().rearrange("p (c n) -> p c n", n=SHARD)
            nc.gpsimd.dma_start(out=m_sb[:, 0:4, :], in_=mt_v[:, 0:4, :])
            nc.gpsimd.dma_start(out=m_sb[:, 4:8, :], in_=mt_v[:, 4:8, :])
            nc.gpsimd.dma_start(out=m_sb[:, 8:12, :], in_=mt_v[:, 8:12, :])
            nc.scalar.dma_start(out=m_sb[:, 12:16, :], in_=mt_v[:, 12:16, :])

            # the replicated-ones block of W: DVE memset, no deps,
            # scheduled right after the preamble.
            nc.vector.memset(w_sb[:, WONE:WONE + B, :], 1.0)

            # short K=1 dummy-matmul bridge: keeps the PE awake (out of the
            # cold p-state) until the first mask group + W block land, with
            # negligible SBUF traffic.  (The hot 2.4GHz state is unreachable
            # in this kernel's ~3us windows - HW DVFS needs sustained real
            # utilization - so the bridge only needs to cover, not warm.)
            wdum = pool.tile([1, 1], BF16)
            nc.vector.memset(wdum, 0.0)
            mdum = pool.tile([1, 512], BF16)
            nc.vector.memset(mdum, 0.0)
            ps_dum = psum_pool.tile([1, 512], F32)
            for _ in range(7):
                nc.tensor.matmul(ps_dum, lhsT=wdum, rhs=mdum, start=True, stop=True)
            for _ in range(4):
                nc.tensor.matmul(ps_dum[:, 0:64], lhsT=wdum, rhs=mdum[:, 0:64],
                                 start=True, stop=True)

            # ---- stationary operand W = [hi | lo | sgn | ones], bf16, in 4
            # pipelined blocks of 4 chunks.  ln|x| = ln(x^2) (x^2 on DVE
            # avoids the Abs table); the 0.5 is folded into the Exp scale.
            # ACT does only the Ln's; DVE everything else.
            sq_sb = pool.tile([128, B, KC], F32)
            ln_sb = pool.tile([128, B, KC], F32)
            for h in range(0, KC, PB):
                sl = slice(h, h + PB)
                nc.vector.tensor_tensor(
                    out=sq_sb[:, :, sl], in0=x_sb[:, :, sl], in1=x_sb[:, :, sl],
                    op=AluOpType.mult)
                nc.scalar.activation(out=ln_sb[:, :, sl], in_=sq_sb[:, :, sl], func=AF.Ln)
                nc.vector.tensor_scalar(
                    out=w_sb[:, WHI:WHI + B, sl], in0=ln_sb[:, :, sl],
                    scalar1=0.0, scalar2=None, op0=AluOpType.add)
                nc.vector.tensor_tensor(
                    out=w_sb[:, WLO:WLO + B, sl], in0=ln_sb[:, :, sl],
                    in1=w_sb[:, WHI:WHI + B, sl], op=AluOpType.subtract)

            # dummy Exp AFTER the Ln phase (input reads ln_sb to pin the
            # ordering): its table load overlaps the matmuls instead of
            # stalling the real Exps.
            dex = pool.tile([1, 1], F32)
            nc.scalar.activation(out=dex, in_=ln_sb[0:1, 0:1, KC - 1], func=AF.Exp)

            # ---- main accumulation: ps[0:128] += W_c^T @ M_c over 16 chunks.
            # Rows 0:32 = Lhi, 32:64 = Llo, 64:96 = C, 96:128 = deg (x32).
            ps = psum_pool.tile([128, SHARD], F32)
            for c in range(KC):
                nc.tensor.matmul(
                    ps, lhsT=w_sb[:, :, c], rhs=m_sb[:, c, :],
                    start=(c == 0), stop=(c == KC - 1))

            # ---- epilogue.  One ACT copy moves the whole PSUM bank to SBUF
            # (PSUM readers serialize per bank - one reader beats four), then:
            #   magh/magl = exp(.5 * csb[0:32 / 32:64])          (ACT)
            #   ci2  = 2*C as int32            (csb[64:96] -> base 96)
            #   odd2 = ci2 & 2                 (base 96)
            #   zv   = min(deg,1) - odd2       (csb[96:128] & odd2, base 96 ->
            #                                   base 0; only INPUT bases must
            #                                   match for 2-tensor DVE ops)
            #   out  = magh*magl*zv
            # deg==0 implies C==0, so empty rows get exactly 0.
            csb = pool.tile([128, SHARD], F32)
            nc.vector.tensor_scalar(
                out=csb, in0=ps, scalar1=0.0, scalar2=None, op0=AluOpType.add)
            ci2 = pool.tile([128, SHARD], I32)
            nc.vector.tensor_scalar(
                out=ci2[WONE:WONE + B, :], in0=csb[WSGN:WSGN + B, :],
                scalar1=2.0, scalar2=None, op0=AluOpType.mult)
            odd2 = pool.tile([128, SHARD], I32)
            nc.vector.tensor_scalar(
                out=odd2[WONE:WONE + B, :], in0=ci2[WONE:WONE + B, :],
                scalar1=2, scalar2=None, op0=AluOpType.bitwise_and)
            zv = pool.tile([B, SHARD], F32)
            nc.vector.scalar_tensor_tensor(
                out=zv, in0=csb[WONE:WONE + B, :], scalar=1.0,
                in1=odd2[WONE:WONE + B, :],
                op0=AluOpType.min, op1=AluOpType.subtract)
            magh = pool.tile([B, SHARD], F32)
            nc.scalar.activation(out=magh, in_=csb[WHI:WHI + B, :], func=AF.Exp, scale=0.5)
            magl = pool.tile([B, SHARD], F32)
            nc.scalar.activation(out=magl, in_=csb[WLO:WLO + B, :], func=AF.Exp, scale=0.5)
            a = pool.tile([B, SHARD], F32)
            nc.vector.tensor_tensor(out=a, in0=magh, in1=magl, op=AluOpType.mult)
            o_sb = pool.tile([B, SHARD], F32)
            nc.vector.tensor_tensor(out=o_sb, in0=a, in1=zv, op=AluOpType.mult)
            # output in two halves on the two HWDGE rings
            H = SHARD // 2
            nc.sync.dma_start(out=out.ap()[:, 0:H], in_=o_sb[:, 0:H])
            nc.scalar.dma_start(out=out.ap()[:, H:SHARD], in_=o_sb[:, H:SHARD])

    nc.compile()
    return nc


def _get_program():
    global _PROG
    if _PROG is None:
        _PROG = _build_program()
    return _PROG


def _prep_inputs(x, mask):
    import ml_dtypes

    x = np.ascontiguousarray(x, dtype=np.float32)
    mask = np.ascontiguousarray(mask, dtype=np.float32)
    # xt[p, b*KC + c] = x[b, c*128 + p]
    xtf = np.ascontiguousarray(
        x.T.reshape(KC, 128, B).transpose(1, 2, 0).reshape(128, B * KC))
    st = (xtf < 0).astype(np.int8)
    xt = xtf.astype(np.float16)
    mask_bf = mask.astype(ml_dtypes.bfloat16)
    in_maps = []
    for k in range(NCORES):
        shard = mask_bf[k * SHARD:(k + 1) * SHARD, :]      # [256, 2048]
        # mt[p, c*SHARD + n] = mask[k*SHARD + n, c*128 + p]
        mt = np.ascontiguousarray(
            shard.T.reshape(KC, 128, SHARD).transpose(1, 0, 2).reshape(128, KC * SHARD))
        in_maps.append({"xt": xt, "st": st, "mt": mt})
    return in_maps


def run(x, mask, trace=False):
    """Run on 8 NeuronCores; returns (output, BassKernelResults)."""
    from concourse.bass_utils import run_bass_kernel_spmd

    nc = _get_program()
    in_maps = _prep_inputs(x, mask)
    res = run_bass_kernel_spmd(nc, in_maps, core_ids=list(range(NCORES)), trace=trace)
    out = np.concatenate([r["out"] for r in res.results], axis=1)
    return np.ascontiguousarray(out, dtype=np.float32), res


def kernel(x, mask):
    out, _ = run(x, mask, trace=False)
    return out


# revision 21
# speedup vs baseline: 1.0516x; 1.0248x over previous
"""Trainium2 Bass kernel for nn_CnUpdateLayer (LDPC check-node update).

Math: out[b,i] = prod_{j: mask[i,j]!=0} x[b,j], or 0 if mask row i is empty.
Since mask is exactly {0,1} and x ~ randn (no exact zeros), the masked product
is computed in log-domain via matmul:

    L[b,i]  = sum_j ln|x[b,j]| * mask[i,j]       (magnitude, log domain)
    C[b,i]  = sum_j [x[b,j]<0] * mask[i,j]       (negative count)
    deg[i]  = sum_j mask[i,j]                    (row degree)
    out     = exp(.5*Lhi)*exp(.5*Llo) * (min(deg,1) - 2*(C&1))

ln(x^2) is split hi/lo into two bf16 halves so the matmul runs at bf16 rate
while keeping ~fp32 accuracy.  The stationary operand is
[ln_hi | ln_lo | signbits | ones*32] = 128 columns: the 32 identical "ones"
columns replicate the row degree into PSUM partitions 96:128, so the epilogue
needs no K=1 broadcast matmul.

Schedule (final):
  - Inputs ride 3 DMA rings: x (fp16) + sign bits + last mask group on the
    sync HWDGE ring, first two mask groups on the SWDGE (gpsimd) ring (it
    sustains 150-230 GB/s vs ~40-80 for each HWDGE ring when all are live),
    third group on the scalar HWDGE ring.  Mask group arrival order matches
    matmul chunk order.  W is stored column-major [128, 128cols, KC] so the
    sign-bit DMA and ones memset write contiguous bytes per partition (a
    strided 64B-element dest shatters the DMA into tiny packets and chokes
    every ring) while each chunk's lhsT stays a single-stride AP.
  - A short K=1 dummy-matmul bridge keeps the PE out of its cold p-state
    until the first mask group lands (the hot 2.4GHz state is unreachable:
    HW DVFS needs sustained real utilization, so matmuls run at 1.2GHz).
  - ln-prep: 4 pipelined blocks; DVE does squares + hi-casts + lo, ACT only
    the 4 Ln's.  GpSimd ALU is never used (~30x slower than DVE and its
    SBUF traffic interferes with DVE).
  - Epilogue: one ACT copy moves all 128 PSUM rows to SBUF (PSUM readers
    serialize per bank, so one reader beats four); int parity (x2 -> int32,
    &2) runs on DVE at partition base 96 so every 2-tensor instruction's
    SBUF inputs share a base (a hardware requirement); output leaves in two
    halves on the two HWDGE rings.
"""

import sys

if "/opt/trn_rl_repo" not in sys.path:
    sys.path.insert(0, "/opt/trn_rl_repo")

import numpy as np

B = 32          # batch codewords
IN_F = 2048     # input edges
OUT_F = 2048    # output edges
NCORES = 8
SHARD = OUT_F // NCORES     # 256 output edges per core
KC = IN_F // 128            # 16 contraction chunks of 128
PB = 4                      # prep block size (chunks)
WHI, WLO, WSGN, WONE = 0, B, 2 * B, 3 * B       # 0, 32, 64, 96
WTOT = 4 * B                                    # 128

NDUM_BIG = 12           # 512-row dummy matmuls (PE p-state warmers)
NDUM_SM = 6             # 64-row dummy matmuls (fine tail trim)

_PROG = None


def _build_program():
    import concourse.tile as tile
    from concourse import bacc, mybir
    from concourse.alu_op_type import AluOpType

    F32 = mybir.dt.float32
    F16 = mybir.dt.float16
    I32 = mybir.dt.int32
    BF16 = mybir.dt.bfloat16
    AF = mybir.ActivationFunctionType

    nc = bacc.Bacc("TRN2", target_bir_lowering=False)
    xt = nc.dram_tensor("xt", [128, KC * B], F16, kind="ExternalInput")
    I8 = mybir.dt.int8
    st = nc.dram_tensor("st", [128, KC * B], I8, kind="ExternalInput")
    mt = nc.dram_tensor("mt", [128, KC * SHARD], BF16, kind="ExternalInput")
    out = nc.dram_tensor("out", [B, SHARD], F32, kind="ExternalOutput")

    with tile.TileContext(nc) as tc:
        with (
            tc.tile_pool(name="pool", bufs=1) as pool,
            tc.tile_pool(name="psum", bufs=1, space="PSUM") as psum_pool,
        ):
            # ---- input DMAs.  x + sign bits on the scalar HWDGE ring (ahead
            # of the Ln table load, which overlaps their transfer); the mask
            # alone on the sync ring in 4 groups of 4 chunks so arrival order
            # matches matmul consumption order.
            # dummy Ln first on the scalar queue: its 1.28us ACT_TABLE_LOAD
            # runs while x streams on the sync ring, and naturally delays the
            # scalar-ring mask issues so x gets the full DMA bandwidth first.
            dmy = pool.tile([1, 1], F32)
            nc.vector.memset(dmy, 1.0)
            dln = pool.tile([1, 1], F32)
            nc.scalar.activation(out=dln, in_=dmy, func=AF.Ln)

            x_sb = pool.tile([128, B, KC], F16)
            nc.sync.dma_start(
                out=x_sb, in_=xt.ap().rearrange("p (b c) -> p b c", c=KC))
            w_sb = pool.tile([128, WTOT, KC], BF16)
            nc.gpsimd.dma_start(
                out=w_sb[:, WSGN:WSGN + B, :],
                in_=st.ap().rearrange("p (b c) -> p b c", c=KC))
            m_sb = pool.tile([128, KC, SHARD], BF16)
            mt_v = mt.ap().rearrange("p (c n) -> p c n", n=SHARD)
            nc.gpsimd.dma_start(out=m_sb[:, 0:4, :], in_=mt_v[:, 0:4, :])
            nc.scalar.dma_start(out=m_sb[:, 4:8, :], in_=mt_v[:, 4:8, :])
            nc.sync.dma_start(out=m_sb[:, 8:12, :], in_=mt_v[:, 8:12, :])
            nc.gpsimd.dma_start(out=m_sb[:, 12:16, :], in_=mt_v[:, 12:16, :])

            # the replicated-ones block of W: DVE memset, no deps,
            # scheduled right after the preamble.
            nc.vector.memset(w_sb[:, WONE:WONE + B, :], 1.0)

            # short K=1 dummy-matmul bridge: keeps the PE awake (out of the
            # cold p-state) until the first mask group + W block land, with
            # negligible SBUF traffic.  (The hot 2.4GHz state is unreachable
            # in this kernel's ~3us windows - HW DVFS needs sustained real
            # utilization - so the bridge only needs to cover, not warm.)
            wdum = pool.tile([1, 1], BF16)
            nc.vector.memset(wdum, 0.0)
            mdum = pool.tile([1, 512], BF16)
            nc.vector.memset(mdum, 0.0)
            ps_dum = psum_pool.tile([1, 512], F32)
            for _ in range(7):
                nc.tensor.matmul(ps_dum, lhsT=wdum, rhs=mdum, start=True, stop=True)
            for _ in range(4):
                nc.tensor.matmul(ps_dum[:, 0:64], lhsT=wdum, rhs=mdum[:, 0:64],
                                 start=True, stop=True)

            # ---- stationary operand W = [hi | lo | sgn | ones], bf16, in 4
            # pipelined blocks of 4 chunks.  ln|x| = ln(x^2) (x^2 on DVE
            # avoids the Abs table); the 0.5 is folded into the Exp scale.
            # ACT does only the Ln's; DVE everything else.
            sq_sb = pool.tile([128, B, KC], F32)
            ln_sb = pool.tile([128, B, KC], F32)
            for h in range(0, KC, PB):
                sl = slice(h, h + PB)
                nc.vector.tensor_tensor(
                    out=sq_sb[:, :, sl], in0=x_sb[:, :, sl], in1=x_sb[:, :, sl],
                    op=AluOpType.mult)
                nc.scalar.activation(out=ln_sb[:, :, sl], in_=sq_sb[:, :, sl], func=AF.Ln)
                nc.vector.tensor_scalar(
                    out=w_sb[:, WHI:WHI + B, sl], in0=ln_sb[:, :, sl],
                    scalar1=0.0, scalar2=None, op0=AluOpType.add)
                nc.vector.tensor_tensor(
                    out=w_sb[:, WLO:WLO + B, sl], in0=ln_sb[:, :, sl],
                    in1=w_sb[:, WHI:WHI + B, sl], op=AluOpType.subtract)

            # dummy Exp AFTER the Ln phase (input reads ln_sb to pin the
            # ordering): its table load overlaps the matmuls instead of
            # stalling the real Exps.
            dex = pool.tile([1, 1], F32)
            nc.scalar.activation(out=dex, in_=ln_sb[0:1, 0:1, KC - 1], func=AF.Exp)

            # ---- main accumulation: ps[0:128] += W_c^T @ M_c over 16 chunks.
            # Rows 0:32 = Lhi, 32:64 = Llo, 64:96 = C, 96:128 = deg (x32).
            ps = psum_pool.tile([128, SHARD], F32)
            for c in range(KC):
                nc.tensor.matmul(
                    ps, lhsT=w_sb[:, :, c], rhs=m_sb[:, c, :],
                    start=(c == 0), stop=(c == KC - 1))

            # ---- epilogue.  One ACT copy moves the whole PSUM bank to SBUF
            # (PSUM readers serialize per bank - one reader beats four), then:
            #   magh/magl = exp(.5 * csb[0:32 / 32:64])          (ACT)
            #   ci2  = 2*C as int32            (csb[64:96] -> base 96)
            #   odd2 = ci2 & 2                 (base 96)
            #   zv   = min(deg,1) - odd2       (csb[96:128] & odd2, base 96 ->
            #                                   base 0; only INPUT bases must
            #                                   match for 2-tensor DVE ops)
            #   out  = magh*magl*zv
            # deg==0 implies C==0, so empty rows get exactly 0.
            csb = pool.tile([128, SHARD], F32)
            nc.vector.tensor_scalar(
                out=csb, in0=ps, scalar1=0.0, scalar2=None, op0=AluOpType.add)
            ci2 = pool.tile([128, SHARD], I32)
            nc.vector.tensor_scalar(
                out=ci2[WONE:WONE + B, :], in0=csb[WSGN:WSGN + B, :],
                scalar1=2.0, scalar2=None, op0=AluOpType.mult)
            odd2 = pool.tile([128, SHARD], I32)
            nc.vector.tensor_scalar(
                out=odd2[WONE:WONE + B, :], in0=ci2[WONE:WONE + B, :],
                scalar1=2, scalar2=None, op0=AluOpType.bitwise_and)
            zv = pool.tile([B, SHARD], F32)
            nc.vector.scalar_tensor_tensor(
                out=zv, in0=csb[WONE:WONE + B, :], scalar=1.0,
                in1=odd2[WONE:WONE + B, :],
                op0=AluOpType.min, op1=AluOpType.subtract)
            magh = pool.tile([B, SHARD], F32)
            nc.scalar.activation(out=magh, in_=csb[WHI:WHI + B, :], func=AF.Exp, scale=0.5)
            magl = pool.tile([B, SHARD], F32)
            nc.scalar.activation(out=magl, in_=csb[WLO:WLO + B, :], func=AF.Exp, scale=0.5)
            a = pool.tile([B, SHARD], F32)
            nc.vector.tensor_tensor(out=a, in0=magh, in1=magl, op=AluOpType.mult)
            o_sb = pool.tile([B, SHARD], F32)
            nc.vector.tensor_tensor(out=o_sb, in0=a, in1=zv, op=AluOpType.mult)
            # output in two halves on the two HWDGE rings
            H = SHARD // 2
            nc.sync.dma_start(out=out.ap()[:, 0:H], in_=o_sb[:, 0:H])
            nc.scalar.dma_start(out=out.ap()[:, H:SHARD], in_=o_sb[:, H:SHARD])

    nc.compile()
    return nc


def _get_program():
    global _PROG
    if _PROG is None:
        _PROG = _build_program()
    return _PROG


def _prep_inputs(x, mask):
    import ml_dtypes

    x = np.ascontiguousarray(x, dtype=np.float32)
    mask = np.ascontiguousarray(mask, dtype=np.float32)
    # xt[p, b*KC + c] = x[b, c*128 + p]
    xtf = np.ascontiguousarray(
        x.T.reshape(KC, 128, B).transpose(1, 2, 0).reshape(128, B * KC))
    st = (xtf < 0).astype(np.int8)
    xt = xtf.astype(np.float16)
    mask_bf = mask.astype(ml_dtypes.bfloat16)
    in_maps = []
    for k in range(NCORES):
        shard = mask_bf[k * SHARD:(k + 1) * SHARD, :]      # [256, 2048]
        # mt[p, c*SHARD + n] = mask[k*SHARD + n, c*128 + p]
        mt = np.ascontiguousarray(
            shard.T.reshape(KC, 128, SHARD).transpose(1, 0, 2).reshape(128, KC * SHARD))
        in_maps.append({"xt": xt, "st": st, "mt": mt})
    return in_maps


def run(x, mask, trace=False):
    """Run on 8 NeuronCores; returns (output, BassKernelResults)."""
    from concourse.bass_utils import run_bass_kernel_spmd

    nc = _get_program()
    in_maps = _prep_inputs(x, mask)
    res = run_bass_kernel_spmd(nc, in_maps, core_ids=list(range(NCORES)), trace=trace)
    out = np.concatenate([r["out"] for r in res.results], axis=1)
    return np.ascontiguousarray(out, dtype=np.float32), res


def kernel(x, mask):
    out, _ = run(x, mask, trace=False)
    return out
